# revision 10
# baseline (speedup 1.0000x reference)
"""Trainium2 Bass kernel for nn_AttentionModule (GNN message passing attention).

Math
----
The reference computes, for N=500000 neighbors (S=128, H=64):
    emb    = ns @ embed_w + embed_b                  [N, H]
    keys   = emb @ key_w + key_b                     [N, H]
    query  = cat(my_state, emb.mean(0)) @ query_w + query_b
    scores = keys @ wk + (query @ wq + attn_b)       [N]
    aw     = softmax(scores);  we = aw @ emb

Softmax is shift-invariant, so every score term that is constant across
neighbors (the whole query path, key_b @ wk, attn_b) cancels.  With
    u = embed_w @ (key_w @ attn_w[H:2H, 0])          [S]
we have aw = softmax(ns @ u) exactly.  The scores are O(+-4) for this data
distribution, so no running-max subtraction is needed (exp stays ~1e-2..4e1).
With e_i = exp(ns_i @ u), Z = sum(e), t = sum_i e_i * ns_i:
    aw = e / Z
    we = (t / Z) @ embed_w + embed_b                 (since sum(aw) == 1)

Device work (the memory-bound part): one streaming pass over ns.
  - raw_i = ns_i . u   : DVE fused multiply+reduce (tensor_tensor_reduce)
  - e = exp(raw)       : ACT
  - t += ns_g^T @ e_g  : PE accumulating matmuls (stationary = ns tile)
  - Z                  : DVE row reduce + PE cross-partition reduce
  - AllReduce([t; Z]) across the 8 cores, then aw = e * (1/Z) and writeback.
Host does only the tiny weight algebra (u, and the final [S]x[S,H] matmul).

Sharding: ns row-sharded 8 ways (62500 rows/core).  Per core, rows are laid
out in a blocked layout: partition p owns rows [p*488, (p+1)*488), plus 36
leftover rows on partitions 0..35.  This makes every DMA fully contiguous
per partition (31 KB lines in, 2 KB lines out).
"""

import numpy as np

N, S, H = 500000, 128, 64
NCORES = 8
R = N // NCORES          # 62500 rows per core
P = 128                  # SBUF partitions
Q = R // P               # 488 rows per partition in the blocked layout
EXTRA = R - P * Q        # 36 leftover rows (live on partitions 0..35)
NT = 8                   # number of streaming DMA tiles
C = Q // NT              # 61 row-groups (of 128 rows) per tile

_cache: dict = {}


def _build(R=R, Q=Q, NT=NT, C=C, EXTRA=EXTRA, compile=True):
    assert R == P * Q + EXTRA and Q == NT * C
    import concourse.bacc as bacc
    import concourse.tile as tile
    from concourse.mybir import dt, AluOpType, AxisListType, ActivationFunctionType

    f32 = dt.float32
    nc = bacc.Bacc("TRN2", target_bir_lowering=False, debug=False,
                   num_devices=NCORES)
    ns = nc.dram_tensor("ns", [R, S], f32, kind="ExternalInput")
    urep = nc.dram_tensor("urep", [P, C * S], f32, kind="ExternalInput")
    aw = nc.dram_tensor("aw", [R], f32, kind="ExternalOutput")
    tz = nc.dram_tensor("tz", [S + 1], f32, kind="ExternalOutput")

    # blocked views: element [p, q, f] = ns[p*Q + q, f]
    ns_blk = ns.ap()[0:P * Q, :].rearrange("(p q) f -> p q f", q=Q)
    aw_blk = aw.ap()[0:P * Q].rearrange("(p q) -> p q", q=Q)

    with tile.TileContext(nc) as tc:
        with tc.tile_pool(name="const", bufs=1) as constp, \
             tc.tile_pool(name="nsp", bufs=2) as nsp, \
             tc.tile_pool(name="scr", bufs=2) as scrp, \
             tc.tile_pool(name="ebp", bufs=NT) as ebp, \
             tc.tile_pool(name="small", bufs=1) as smp, \
             tc.tile_pool(name="ps", bufs=1, space="PSUM") as psp, \
             tc.tile_pool(name="dram", bufs=1, space="DRAM") as drp:

            u_sb = constp.tile([P, C * S], f32)
            nc.sync.dma_start(u_sb[:, :], urep.ap())
            ones_col = constp.tile([P, 1], f32)
            nc.gpsimd.memset(ones_col[:, :], 1.0)
            ones_row = constp.tile([1, P], f32)
            nc.gpsimd.memset(ones_row[:, :], 1.0)

            t_ps = psp.tile([P, 1], f32)     # t accumulator (PSUM)
            z_ps = psp.tile([1, 1], f32)     # Z scalar (PSUM)
            b_ps = psp.tile([P, 1], f32)     # 1/Z broadcast (PSUM)

            # DVE multiply chunking: split each tile's big elementwise
            # multiply so the ACT accumulation can start before the whole
            # tile's product is done.
            NCH = 4
            chunk = (C + NCH - 1) // NCH

            e_tiles = []
            # --- streaming pass over ns ---
            for i in range(NT):
                tl = nsp.tile([P, C * S], f32, tag="ns")
                nc.sync.dma_start(tl[:, :], ns_blk[:, i * C:(i + 1) * C, :])
                e_i = ebp.tile([P, C], f32, tag=f"e{i}")
                e_tiles.append(e_i)
                prod = scrp.tile([P, C * S], f32, tag="prod")
                for k in range(0, C, chunk):
                    ke = min(k + chunk, C)
                    nc.vector.tensor_tensor(
                        prod[:, k * S:ke * S], tl[:, k * S:ke * S],
                        u_sb[:, k * S:ke * S], AluOpType.mult)
                for j in range(C):
                    nc.scalar.activation(
                        prod[:, j * S:(j + 1) * S], prod[:, j * S:(j + 1) * S],
                        ActivationFunctionType.Copy,
                        accum_out=e_i[:, j:j + 1])
                nc.scalar.activation(e_i[:, :], e_i[:, :],
                                     ActivationFunctionType.Exp)
                for j in range(C):
                    g = i * C + j
                    nc.tensor.matmul(
                        t_ps[:, :],
                        lhsT=tl[:, j * S:(j + 1) * S],
                        rhs=e_i[:, j:j + 1],
                        start=(g == 0), stop=False)

            # --- leftover EXTRA rows on partitions 0..EXTRA-1 ---
            ex = nsp.tile([P, S], f32, tag="ex")
            nc.sync.dma_start(ex[0:EXTRA, :], ns.ap()[P * Q:R, :])
            e_ex = ebp.tile([P, 1], f32, tag="eex")
            nc.gpsimd.memset(e_ex[:, :], 0.0)
            sc = scrp.tile([P, S], f32, tag="sc")
            nc.vector.tensor_tensor(sc[0:EXTRA, :], ex[0:EXTRA, :],
                                    u_sb[0:EXTRA, 0:S], AluOpType.mult)
            nc.scalar.activation(sc[0:EXTRA, :], sc[0:EXTRA, :],
                                 ActivationFunctionType.Copy,
                                 accum_out=e_ex[0:EXTRA, 0:1])
            nc.scalar.activation(e_ex[0:EXTRA, 0:1], e_ex[0:EXTRA, 0:1],
                                 ActivationFunctionType.Exp)
            nc.tensor.matmul(t_ps[:, :], lhsT=ex[0:EXTRA, :],
                             rhs=e_ex[0:EXTRA, 0:1], start=False, stop=True)

            # --- Z = sum(e) ---
            zcols = smp.tile([P, NT + 1], f32)
            for i in range(NT):
                nc.vector.tensor_reduce(zcols[:, i:i + 1], e_tiles[i][:, :],
                                        axis=AxisListType.X, op=AluOpType.add)
            nc.vector.tensor_copy(zcols[:, NT:NT + 1], e_ex[:, :])
            zpart = smp.tile([P, 1], f32)
            nc.vector.tensor_reduce(zpart[:, :], zcols[:, :],
                                    axis=AxisListType.X, op=AluOpType.add)
            nc.tensor.matmul(z_ps[:, :], lhsT=zpart[:, :], rhs=ones_col[:, :],
                             start=True, stop=True)

            # --- all-reduce [t; Z] across the 8 cores ---
            t_sb = smp.tile([P, 1], f32)
            nc.scalar.copy(t_sb[:, :], t_ps[:, :])
            z_sb = smp.tile([1, 1], f32)
            nc.vector.tensor_copy(z_sb[:, :], z_ps[:, :])
            tzb_in = drp.tile([1, S + 1], f32, tag="tzi")
            tzb_out = drp.tile([1, S + 1], f32, tag="tzo")
            nc.sync.dma_start(tzb_in[0:1, 0:S], t_sb[:, :])
            nc.sync.dma_start(tzb_in[0:1, S:S + 1], z_sb[:, :])
            nc.gpsimd.collective_compute(
                "AllReduce", AluOpType.add,
                replica_groups=[list(range(NCORES))],
                ins=[tzb_in[0:1, :].opt()], outs=[tzb_out[0:1, :].opt()])
            nc.sync.dma_start(tz.ap(), tzb_out[0:1, :])

            # --- broadcast 1/Z to all partitions ---
            zt = smp.tile([1, 1], f32)
            nc.sync.dma_start(zt[:, :], tzb_out[0:1, S:S + 1])
            nc.vector.reciprocal(zt[:, :], zt[:, :])
            nc.tensor.matmul(b_ps[:, :], lhsT=ones_row[:, :], rhs=zt[:, :],
                             start=True, stop=True)
            rb = smp.tile([P, 1], f32)
            nc.scalar.copy(rb[:, :], b_ps[:, :])

            # --- aw = e * (1/Z), written back through a staging tile ---
            stage = smp.tile([P, Q], f32)
            for i in range(NT):
                nc.vector.tensor_scalar_mul(stage[:, i * C:(i + 1) * C],
                                            e_tiles[i][:, :], rb[:, :])
            nc.sync.dma_start(aw_blk, stage[:, :])
            st_ex = smp.tile([P, 1], f32)
            nc.vector.tensor_scalar_mul(st_ex[:, :], e_ex[:, :], rb[:, :])
            nc.sync.dma_start(aw.ap()[P * Q:R], st_ex[0:EXTRA, 0:1])

    if compile:
        nc.compile()
    return nc


def _get_nc():
    if "nc" not in _cache:
        _cache["nc"] = _build()
    return _cache["nc"]


def _host_prep(inputs):
    ns_full = np.ascontiguousarray(
        np.asarray(inputs["neighbor_states"], dtype=np.float32))
    embed_w = np.asarray(inputs["embed_w"], dtype=np.float32)
    embed_b = np.asarray(inputs["embed_b"], dtype=np.float32)
    key_w = np.asarray(inputs["key_w"], dtype=np.float32)
    attn_w = np.asarray(inputs["attn_w"], dtype=np.float32)
    wk = attn_w[H:2 * H, 0]
    u = (embed_w.astype(np.float64) @ (key_w.astype(np.float64) @
                                       wk.astype(np.float64)))
    urep_arr = np.ascontiguousarray(
        np.broadcast_to(np.tile(u.astype(np.float32), C), (P, C * S)))
    in_maps = [{"ns": ns_full[c * R:(c + 1) * R], "urep": urep_arr}
               for c in range(NCORES)]
    return in_maps, embed_w, embed_b


def _run(inputs, trace=False):
    from concourse.bass_utils import run_bass_kernel_spmd
    nc = _get_nc()
    in_maps, embed_w, embed_b = _host_prep(inputs)
    res = run_bass_kernel_spmd(nc, in_maps, core_ids=list(range(NCORES)),
                               trace=trace)
    aw_out = np.concatenate([res.results[c]["aw"] for c in range(NCORES)])
    tzv = res.results[0]["tz"]
    t, z = tzv[:S], tzv[S]
    we = ((t / z) @ embed_w + embed_b).astype(np.float32)
    return (we, aw_out.astype(np.float32)), res


def kernel(**inputs):
    out, _ = _run(inputs, trace=False)
    return out


# revision 12
# speedup vs baseline: 1.5499x; 1.5499x over previous
"""Trainium2 Bass kernel for nn_AttentionModule (GNN message passing attention).

Math
----
The reference computes, for N=500000 neighbors (S=128, H=64):
    emb    = ns @ embed_w + embed_b                  [N, H]
    keys   = emb @ key_w + key_b                     [N, H]
    query  = cat(my_state, emb.mean(0)) @ query_w + query_b
    scores = keys @ wk + (query @ wq + attn_b)       [N]
    aw     = softmax(scores);  we = aw @ emb

Softmax is shift-invariant, so every score term that is constant across
neighbors (the whole query path, key_b @ wk, attn_b) cancels.  With
    u = embed_w @ (key_w @ attn_w[H:2H, 0])          [S]
we have aw = softmax(ns @ u) exactly.  The scores are O(+-4) for this data
distribution, so no running-max subtraction is needed (exp stays ~1e-2..4e1).
With e_i = exp(ns_i @ u), Z = sum(e), t = sum_i e_i * ns_i:
    aw = e / Z
    we = (t / Z) @ embed_w + embed_b                 (since sum(aw) == 1)

Device work (the memory-bound part): one streaming pass over ns.
  - raw_i = ns_i . u   : DVE fused multiply+reduce (tensor_tensor_reduce)
  - e = exp(raw)       : ACT
  - t += ns_g^T @ e_g  : PE accumulating matmuls (stationary = ns tile)
  - Z                  : DVE row reduce + PE cross-partition reduce
  - AllReduce([t; Z]) across the 8 cores, then aw = e * (1/Z) and writeback.
Host does only the tiny weight algebra (u, and the final [S]x[S,H] matmul).

Sharding: ns row-sharded 8 ways (62500 rows/core).  Per core, rows are laid
out in a blocked layout: partition p owns rows [p*488, (p+1)*488), plus 36
leftover rows on partitions 0..35.  This makes every DMA fully contiguous
per partition (31 KB lines in, 2 KB lines out).
"""

import numpy as np

N, S, H = 500000, 128, 64
NCORES = 8
R = N // NCORES          # 62500 rows per core
P = 128                  # SBUF partitions
Q = R // P               # 488 rows per partition in the blocked layout
EXTRA = R - P * Q        # 36 leftover rows (live on partitions 0..35)
NT = 8                   # number of streaming DMA tiles
C = Q // NT              # 61 row-groups (of 128 rows) per tile

_cache: dict = {}


def _build(R=R, Q=Q, NT=NT, C=C, EXTRA=EXTRA, compile=True):
    assert R == P * Q + EXTRA and Q == NT * C
    import concourse.bacc as bacc
    import concourse.tile as tile
    from concourse.mybir import dt, AluOpType, AxisListType, ActivationFunctionType

    f32 = dt.float32
    nc = bacc.Bacc("TRN2", target_bir_lowering=False, debug=False,
                   num_devices=NCORES)
    ns = nc.dram_tensor("ns", [R, S], f32, kind="ExternalInput")
    urep = nc.dram_tensor("urep", [P, C * S], f32, kind="ExternalInput")
    aw = nc.dram_tensor("aw", [R], f32, kind="ExternalOutput")
    tz = nc.dram_tensor("tz", [S + 1], f32, kind="ExternalOutput")

    # blocked views: element [p, q, f] = ns[p*Q + q, f]
    ns_blk = ns.ap()[0:P * Q, :].rearrange("(p q) f -> p q f", q=Q)
    aw_blk = aw.ap()[0:P * Q].rearrange("(p q) -> p q", q=Q)

    with tile.TileContext(nc) as tc:
        with tc.tile_pool(name="const", bufs=1) as constp, \
             tc.tile_pool(name="nsp", bufs=2) as nsp, \
             tc.tile_pool(name="scr", bufs=2) as scrp, \
             tc.tile_pool(name="ebp", bufs=NT) as ebp, \
             tc.tile_pool(name="small", bufs=1) as smp, \
             tc.tile_pool(name="ps", bufs=1, space="PSUM") as psp, \
             tc.tile_pool(name="dram", bufs=1, space="DRAM") as drp:

            u_sb = constp.tile([P, C * S], f32)
            nc.sync.dma_start(u_sb[:, :], urep.ap())
            ones_col = constp.tile([P, 1], f32)
            nc.gpsimd.memset(ones_col[:, :], 1.0)
            ones_row = constp.tile([1, P], f32)
            nc.gpsimd.memset(ones_row[:, :], 1.0)

            t_ps = psp.tile([P, 1], f32)     # t accumulator (PSUM)
            z_ps = psp.tile([1, 1], f32)     # Z scalar (PSUM)
            b_ps = psp.tile([P, 1], f32)     # 1/Z broadcast (PSUM)

            # DVE multiply chunking: split each tile's big elementwise
            # multiply so the ACT accumulation can start before the whole
            # tile's product is done.
            NCH = 4
            chunk = (C + NCH - 1) // NCH

            e_tiles = []
            # --- streaming pass over ns ---
            for i in range(NT):
                tl = nsp.tile([P, C * S], f32, tag="ns")
                nc.sync.dma_start(tl[:, :], ns_blk[:, i * C:(i + 1) * C, :])
                e_i = ebp.tile([P, C], f32, tag=f"e{i}")
                e_tiles.append(e_i)
                prod = scrp.tile([P, C * S], f32, tag="prod")
                for k in range(0, C, chunk):
                    ke = min(k + chunk, C)
                    nc.vector.tensor_tensor(
                        prod[:, k * S:ke * S], tl[:, k * S:ke * S],
                        u_sb[:, k * S:ke * S], AluOpType.mult)
                    nc.vector.tensor_reduce(
                        e_i[:, k:ke],
                        prod[:, k * S:ke * S].rearrange("p (c s) -> p c s", s=S),
                        axis=AxisListType.X, op=AluOpType.add)
                nc.scalar.activation(e_i[:, :], e_i[:, :],
                                     ActivationFunctionType.Exp)
                for j in range(C):
                    g = i * C + j
                    nc.tensor.matmul(
                        t_ps[:, :],
                        lhsT=tl[:, j * S:(j + 1) * S],
                        rhs=e_i[:, j:j + 1],
                        start=(g == 0), stop=False)

            # --- leftover EXTRA rows on partitions 0..EXTRA-1 ---
            ex = nsp.tile([P, S], f32, tag="ex")
            nc.sync.dma_start(ex[0:EXTRA, :], ns.ap()[P * Q:R, :])
            e_ex = ebp.tile([P, 1], f32, tag="eex")
            nc.gpsimd.memset(e_ex[:, :], 0.0)
            sc = scrp.tile([P, S], f32, tag="sc")
            nc.vector.tensor_tensor(sc[0:EXTRA, :], ex[0:EXTRA, :],
                                    u_sb[0:EXTRA, 0:S], AluOpType.mult)
            nc.vector.tensor_reduce(e_ex[0:EXTRA, 0:1], sc[0:EXTRA, :],
                                    axis=AxisListType.X, op=AluOpType.add)
            nc.scalar.activation(e_ex[0:EXTRA, 0:1], e_ex[0:EXTRA, 0:1],
                                 ActivationFunctionType.Exp)
            nc.tensor.matmul(t_ps[:, :], lhsT=ex[0:EXTRA, :],
                             rhs=e_ex[0:EXTRA, 0:1], start=False, stop=True)

            # --- Z = sum(e) ---
            zcols = smp.tile([P, NT + 1], f32)
            for i in range(NT):
                nc.vector.tensor_reduce(zcols[:, i:i + 1], e_tiles[i][:, :],
                                        axis=AxisListType.X, op=AluOpType.add)
            nc.vector.tensor_copy(zcols[:, NT:NT + 1], e_ex[:, :])
            zpart = smp.tile([P, 1], f32)
            nc.vector.tensor_reduce(zpart[:, :], zcols[:, :],
                                    axis=AxisListType.X, op=AluOpType.add)
            nc.tensor.matmul(z_ps[:, :], lhsT=zpart[:, :], rhs=ones_col[:, :],
                             start=True, stop=True)

            # --- all-reduce [t; Z] across the 8 cores ---
            t_sb = smp.tile([P, 1], f32)
            nc.scalar.copy(t_sb[:, :], t_ps[:, :])
            z_sb = smp.tile([1, 1], f32)
            nc.vector.tensor_copy(z_sb[:, :], z_ps[:, :])
            tzb_in = drp.tile([1, S + 1], f32, tag="tzi")
            tzb_out = drp.tile([1, S + 1], f32, tag="tzo")
            nc.sync.dma_start(tzb_in[0:1, 0:S], t_sb[:, :])
            nc.sync.dma_start(tzb_in[0:1, S:S + 1], z_sb[:, :])
            nc.gpsimd.collective_compute(
                "AllReduce", AluOpType.add,
                replica_groups=[list(range(NCORES))],
                ins=[tzb_in[0:1, :].opt()], outs=[tzb_out[0:1, :].opt()])
            nc.sync.dma_start(tz.ap(), tzb_out[0:1, :])

            # --- broadcast 1/Z to all partitions ---
            zt = smp.tile([1, 1], f32)
            nc.sync.dma_start(zt[:, :], tzb_out[0:1, S:S + 1])
            nc.vector.reciprocal(zt[:, :], zt[:, :])
            nc.tensor.matmul(b_ps[:, :], lhsT=ones_row[:, :], rhs=zt[:, :],
                             start=True, stop=True)
            rb = smp.tile([P, 1], f32)
            nc.scalar.copy(rb[:, :], b_ps[:, :])

            # --- aw = e * (1/Z), written back through a staging tile ---
            stage = smp.tile([P, Q], f32)
            for i in range(NT):
                nc.vector.tensor_scalar_mul(stage[:, i * C:(i + 1) * C],
                                            e_tiles[i][:, :], rb[:, :])
            nc.sync.dma_start(aw_blk, stage[:, :])
            st_ex = smp.tile([P, 1], f32)
            nc.vector.tensor_scalar_mul(st_ex[:, :], e_ex[:, :], rb[:, :])
            nc.sync.dma_start(aw.ap()[P * Q:R], st_ex[0:EXTRA, 0:1])

    if compile:
        nc.compile()
    return nc


def _get_nc():
    if "nc" not in _cache:
        _cache["nc"] = _build()
    return _cache["nc"]


def _host_prep(inputs):
    ns_full = np.ascontiguousarray(
        np.asarray(inputs["neighbor_states"], dtype=np.float32))
    embed_w = np.asarray(inputs["embed_w"], dtype=np.float32)
    embed_b = np.asarray(inputs["embed_b"], dtype=np.float32)
    key_w = np.asarray(inputs["key_w"], dtype=np.float32)
    attn_w = np.asarray(inputs["attn_w"], dtype=np.float32)
    wk = attn_w[H:2 * H, 0]
    u = (embed_w.astype(np.float64) @ (key_w.astype(np.float64) @
                                       wk.astype(np.float64)))
    urep_arr = np.ascontiguousarray(
        np.broadcast_to(np.tile(u.astype(np.float32), C), (P, C * S)))
    in_maps = [{"ns": ns_full[c * R:(c + 1) * R], "urep": urep_arr}
               for c in range(NCORES)]
    return in_maps, embed_w, embed_b


def _run(inputs, trace=False):
    from concourse.bass_utils import run_bass_kernel_spmd
    nc = _get_nc()
    in_maps, embed_w, embed_b = _host_prep(inputs)
    res = run_bass_kernel_spmd(nc, in_maps, core_ids=list(range(NCORES)),
                               trace=trace)
    aw_out = np.concatenate([res.results[c]["aw"] for c in range(NCORES)])
    tzv = res.results[0]["tz"]
    t, z = tzv[:S], tzv[S]
    we = ((t / z) @ embed_w + embed_b).astype(np.float32)
    return (we, aw_out.astype(np.float32)), res


def kernel(**inputs):
    out, _ = _run(inputs, trace=False)
    return out
